# revision 10
# baseline (speedup 1.0000x reference)
"""AntiAliasInterpolation2d (depthwise 13x13 gaussian blur + 0.25x bilinear
downsample) as a single stride-4 14x14 depthwise conv, run SPMD on 8 TRN2
NeuronCores (batch-parallel: 4 images per core).

Math: with SCALE=0.25 the bilinear resample picks rows/cols {4k+1, 4k+2} with
weight 0.5 each, so
    out[i,j] = sum_{m,n=0..13} W2[m,n] * x[4i+m-5, 4j+n-5]     (OOB -> 0)
where W2 = 0.25 * (W + W>>row + W>>col + W>>rowcol) is the 13x13 kernel
convolved with a 2x2 box.  W2 is decomposed per-channel by SVD into rank-1
terms (the gaussian is exactly rank 1), each term a 14-tap row filter `a` and
col filter `b`.

Per core, per channel c (4 images b=0..3):
  pass1 (TensorE, fp32r): tmp2[i,w] = sum_r Ut[r,i] * x[r,w]   (banded Ut)
  transpose tmp2 chunks on TensorE (bf16) -> t2T[w,(b,i)]
  pass2 (TensorE, bf16):  psum[j,(b,i)] = sum_w V[w,j] * t2T[w,(b,i)]
The device emits out_t[c,j,b,i]; the host permutes back to [b,c,i,j].
"""

import os

import numpy as np

os.environ.setdefault("MYCRO_LOCAL_CACHE", "1")

N_CORES = 8
NB = 4  # batch shard per core (32 / 8)
C = 3
H = W = 512
OH = OW = 128
KS = 14  # combined stride-4 kernel size
PAD = 5  # x row index = 4*i + m - PAD

LAST = None  # BassKernelResults of the most recent run (for test harness)

_graph_cache = {}


def _filters(weight):
    """Per-channel rank-1 decomposition of the combined 14x14 kernel."""
    Wk = np.asarray(weight, dtype=np.float64).reshape(C, 13, 13)
    W2 = np.zeros((C, KS, KS))
    W2[:, :13, :13] += Wk
    W2[:, 1:, :13] += Wk
    W2[:, :13, 1:] += Wk
    W2[:, 1:, 1:] += Wk
    W2 *= 0.25
    terms = []
    for c in range(C):
        U, S, Vt = np.linalg.svd(W2[c])
        keep = [r for r in range(KS) if S[r] > max(S[0] * 1e-7, 0.0)]
        terms.append([(U[:, r] * S[r], Vt[r].copy()) for r in keep])
    return terms


def _banded(vec):
    """[512, 128] banded matrix M[xr, i] = vec[xr - 4i + PAD] (in-band only)."""
    M = np.zeros((H, OH), np.float32)
    for i in range(OH):
        base = 4 * i - PAD
        for m in range(KS):
            xr = base + m
            if 0 <= xr < H:
                M[xr, i] = vec[m]
    return M


def _host_tensors(terms):
    """Build the flat banded-matrix stacks shipped to the device.

    ub chunk j is Ut[j::4] (row-residue chunking): the x tile stores rows
    4p..4p+3 on partition p (one contiguous 8KB DMA run per partition), so
    K-chunk j of the contraction is x rows {4p+j} paired with Ut[4p+j, i].
    vb stays block-chunked (it contracts over transposed 128-col blocks).
    """
    ub_chunks, vb_chunks = [], []
    index = {}   # (c, r, ch) -> chunk slot (deduped across identical filters)
    seen = {}    # filter fingerprint -> first slot of its 4 chunks
    for c in range(C):
        for r, (a_vec, b_vec) in enumerate(terms[c]):
            fp = (a_vec.tobytes(), b_vec.tobytes())
            if fp in seen:
                base = seen[fp]
                for ch in range(4):
                    index[(c, r, ch)] = base + ch
                continue
            base = len(ub_chunks)
            seen[fp] = base
            Ua = _banded(a_vec)
            Vb = _banded(b_vec)
            for ch in range(4):
                ub_chunks.append(Ua[ch::4])
                vb_chunks.append(Vb[ch * 128:(ch + 1) * 128])
                index[(c, r, ch)] = base + ch
    ub = np.stack(ub_chunks, axis=1).reshape(128, -1).copy()
    vb = np.stack(vb_chunks, axis=1).reshape(128, -1).copy()
    ident = np.eye(128, dtype=np.float32)
    return ub, vb, ident, index


def _build_graph(ranks, nm, offs):
    """Build the per-core Bass graph. `ranks[c]` = #rank-1 terms for channel c;
    `offs` maps (c, r, chunk) -> deduped chunk slot in ub/vb."""
    import concourse.mybir as mybir
    from concourse import bacc, tile

    f32 = mybir.dt.float32
    COPY = mybir.ActivationFunctionType.Copy

    st_dt = mybir.dt.bfloat16
    bf16 = mybir.dt.bfloat16

    nc = bacc.Bacc()
    # x ships as bf16: halves HBM traffic (the kernel is DMA-bound) and
    # runs pass-1 fully in bf16 (FWL weight loads)
    x = nc.declare_dram_parameter("x", [NB, C, H, W], bf16, isOutput=False)
    ub = nc.declare_dram_parameter("ub", [128, nm * 128], bf16, isOutput=False)
    vb = nc.declare_dram_parameter("vb", [128, nm * 128], st_dt, isOutput=False)
    ident = nc.declare_dram_parameter("ident", [128, 128], st_dt, isOutput=False)
    out = nc.declare_dram_parameter("out", [C, OH, NB, OW], st_dt, isOutput=True)

    max_r = max(ranks)

    with tile.TileContext(nc) as tc:
        with (
            tc.tile_pool(name="const", bufs=1) as constp,
            tc.tile_pool(name="xg", bufs=12) as xpool,
            tc.tile_pool(name="t2", bufs=3) as t2pool,
            tc.tile_pool(name="t2T", bufs=2) as tTpool,
            tc.tile_pool(name="oT", bufs=2) as opool,
            tc.tile_pool(name="ps1", bufs=3, space="PSUM") as ps1pool,
            tc.tile_pool(name="psT", bufs=3, space="PSUM") as psTpool,
            tc.tile_pool(name="ps2", bufs=2, space="PSUM") as ps2pool,
        ):
            # consts split across the two HWDGE rings (sync + scalar) so
            # both start streaming immediately
            ubt = constp.tile([128, nm * 128], bf16)
            nc.sync.dma_start(ubt[:], ub[:])
            vbt = constp.tile([128, nm * 128], st_dt)
            nc.scalar.dma_start(vbt[:], vb[:])
            idt = constp.tile([128, 128], st_dt)
            nc.scalar.dma_start(idt[:], ident[:])

            # plane list: (c, b, r) in processing order, grouped by channel
            planes = [(c, b, r) for c in range(C)
                      for b in range(NB) for r in range(ranks[c])]
            n_pl = len(planes)

            # stage state carried across the software pipeline
            xg = {}     # (c, b) -> x tile
            t2s = {}    # idx -> t2 tile
            psTs = {}   # idx -> psT tile
            t2Ts = {}   # c -> t2T group tile
            ps1s = {}   # idx -> ps1 tile

            def s0_dma(idx):  # input DMA, alternating the two HWDGE rings
                c, b, r = planes[idx]
                if (c, b) in xg:
                    return
                xt = xpool.tile([128, 4, W], bf16, tag="xg")
                eng = nc.sync if (len(xg) % 2 == 0) else nc.scalar
                eng.dma_start(
                    xt[:], x[b, c].rearrange("(p k) w -> p k w", k=4)
                )
                xg[(c, b)] = xt

            def s1_mm(idx):  # pass-1 matmuls (PE)
                c, b, r = planes[idx]
                ps1 = ps1pool.tile([128, W], f32, tag="ps1")
                for kc in range(4):
                    o_ = offs[(c, r, kc)] * 128
                    nc.tensor.matmul(
                        ps1[:],
                        ubt[:, o_:o_ + 128],
                        xg[(c, b)][:, kc, :],
                        start=(kc == 0),
                        stop=(kc == 3),
                    )
                ps1s[idx] = ps1

            def s2_copy(idx):  # PSUM -> SBUF bf16 cast (DVE, otherwise idle)
                t2 = t2pool.tile([128, W], st_dt, tag="t2")
                nc.vector.tensor_copy(t2[:], ps1s.pop(idx)[:])
                t2s[idx] = t2

            def s3_transpose(idx):  # PE transposes
                psT = psTpool.tile([128, W], st_dt, tag="psT")
                t2 = t2s.pop(idx)
                for wc in range(4):
                    s = wc * 128
                    nc.tensor.transpose(psT[:, s:s + 128], t2[:, s:s + 128], idt[:])
                psTs[idx] = psT

            def s4_copy(idx):  # PSUM -> grouped SBUF (ACT)
                c, b, r = planes[idx]
                if c not in t2Ts:
                    t2Ts[c] = tTpool.tile([128, max_r, 4, NB, OH], st_dt, tag="t2T", name=f"t2T_{c}")
                nc.scalar.activation(t2Ts[c][:, r, :, b, :], psTs.pop(idx)[:], COPY)

            def s5_chunk(c, b0, nb, ring=None):
                # pass-2 for images [b0, b0+nb) of channel c
                rc_n = ranks[c]
                bs = slice(b0, b0 + nb)
                ps2 = ps2pool.tile([128, nb * OH], f32, tag="ps2", name=f"ps2_{c}_{b0}")
                n_acc = rc_n * 4
                k2 = 0
                for r in range(rc_n):
                    for wc in range(4):
                        o_ = offs[(c, r, wc)] * 128
                        nc.tensor.matmul(
                            ps2[:],
                            vbt[:, o_:o_ + 128],
                            t2Ts[c][:, r, wc, bs, :],
                            start=(k2 == 0),
                            stop=(k2 == n_acc - 1),
                        )
                        k2 += 1
                if b0 + nb == NB:
                    t2Ts.pop(c)
                oT = opool.tile([128, nb * OH], st_dt, tag="oT", name=f"oT_{c}_{b0}")
                nc.scalar.activation(oT[:], ps2[:], COPY)
                (ring or nc.scalar).dma_start(
                    out[c][:, bs, :], oT[:].rearrange("j (b i) -> j b i", b=nb)
                )

            # warm the PE HAM clock-gate with throwaway matmuls while the
            # input stream fills.  The weights come from a memset tile, not
            # a DMA'd const, so the warmup starts at sequencer boot instead
            # of waiting ~3.5us for the first DMA completion.
            warm = constp.tile([128, 512], bf16)
            nc.gpsimd.memset(warm[:], 0.5)
            wps = ps1pool.tile([128, 512], f32, tag="ps1", name="wps")
            for _ in range(12):
                nc.tensor.matmul(wps[:], warm[:, 0:128], warm[:],
                                 start=True, stop=True)

            # prefetch all input DMAs up front (12 bufs cover all planes)
            done_dma = set()
            for idx in range(n_pl):
                c, b, r = planes[idx]
                if (c, b) not in done_dma:
                    s0_dma(idx)
                    done_dma.add((c, b))

            # software pipeline: keep the PE stream dense
            LAG = 1
            # plane idx -> (c, b0, nb, ring) pass-2 chunk ready at this plane
            half_last = {}
            seen = {}
            for idx, (c, b, r) in enumerate(planes):
                seen.setdefault(c, []).append(idx)
            last_c = max(seen)
            for c, idxs in seen.items():
                rc_n = ranks[c]
                half_last[idxs[2 * rc_n - 1]] = [(c, 0, 2, None)]
                if c == last_c:
                    # final channel: split the tail chunk so the last
                    # arriving image has the shortest chain to HBM, and
                    # send the final output on the (by then idle) sync ring
                    half_last[idxs[3 * rc_n - 1]] = [(c, 2, 1, None)]
                    half_last[idxs[4 * rc_n - 1]] = [(c, 3, 1, nc.sync)]
                else:
                    half_last[idxs[4 * rc_n - 1]] = [(c, 2, 2, None)]
            for idx in range(n_pl + LAG + 1):
                if idx < n_pl:
                    s1_mm(idx)
                if 0 <= idx - LAG < n_pl:
                    j = idx - LAG
                    s2_copy(j)
                    s3_transpose(j)
                    s4_copy(j)
                    for chunk in half_last.get(j, ()):
                        s5_chunk(*chunk)

    nc.compile()
    return nc


def _build_graph_general(ranks, nm, offs):
    """Fallback for high-rank (non-separable) kernels: bf16 end-to-end,
    rank-major loop, small buffers. Correctness over speed — the shipped
    Gaussian weight is rank-1 per channel and never takes this path."""
    import concourse.mybir as mybir
    from concourse import bacc, tile

    f32 = mybir.dt.float32
    bf16 = mybir.dt.bfloat16
    COPY = mybir.ActivationFunctionType.Copy

    nc = bacc.Bacc()
    x = nc.declare_dram_parameter("x", [NB, C, H, W], f32, isOutput=False)
    ub = nc.declare_dram_parameter("ub", [128, nm * 128], bf16, isOutput=False)
    vb = nc.declare_dram_parameter("vb", [128, nm * 128], bf16, isOutput=False)
    ident = nc.declare_dram_parameter("ident", [128, 128], bf16, isOutput=False)
    out = nc.declare_dram_parameter("out", [C, OH, NB, OW], f32, isOutput=True)

    with tile.TileContext(nc) as tc:
        with (
            tc.tile_pool(name="const", bufs=1) as constp,
            tc.tile_pool(name="xg", bufs=6) as xpool,
            tc.tile_pool(name="xb", bufs=6) as xbpool,
            tc.tile_pool(name="t2", bufs=3) as t2pool,
            tc.tile_pool(name="t2T", bufs=3) as tTpool,
            tc.tile_pool(name="oT", bufs=2) as opool,
            tc.tile_pool(name="ps1", bufs=2, space="PSUM") as ps1pool,
            tc.tile_pool(name="psT", bufs=2, space="PSUM") as psTpool,
            tc.tile_pool(name="ps2", bufs=2, space="PSUM") as ps2pool,
        ):
            ubt = constp.tile([128, nm * 128], bf16)
            nc.sync.dma_start(ubt[:], ub[:])
            vbt = constp.tile([128, nm * 128], bf16)
            nc.sync.dma_start(vbt[:], vb[:])
            idt = constp.tile([128, 128], bf16)
            nc.sync.dma_start(idt[:], ident[:])

            for c in range(C):
                rc_n = ranks[c]
                xb = []
                for b in range(NB):
                    xt = xpool.tile([128, 4, W], f32, tag="xg")
                    nc.sync.dma_start(
                        xt[:], x[b, c].rearrange("(p k) w -> p k w", k=4)
                    )
                    xc = xbpool.tile([128, 4, W], bf16, tag="xb")
                    nc.vector.tensor_copy(xc[:], xt[:])
                    xb.append(xc)

                ps2 = ps2pool.tile([128, NB * OH], f32, tag="ps2")
                n_acc = rc_n * 4
                k2 = 0
                for r in range(rc_n):
                    t2T = tTpool.tile([128, 4, NB, OH], bf16, tag="t2T",
                                      name=f"t2Tg_{c}_{r}")
                    for b in range(NB):
                        ps1 = ps1pool.tile([128, W], f32, tag="ps1")
                        for kc in range(4):
                            o_ = offs[(c, r, kc)] * 128
                            nc.tensor.matmul(
                                ps1[:],
                                ubt[:, o_:o_ + 128],
                                xb[b][:, kc, :],
                                start=(kc == 0),
                                stop=(kc == 3),
                            )
                        t2 = t2pool.tile([128, W], bf16, tag="t2")
                        nc.scalar.activation(t2[:], ps1[:], COPY)
                        psT = psTpool.tile([128, W], bf16, tag="psT")
                        for wc in range(4):
                            s = wc * 128
                            nc.tensor.transpose(
                                psT[:, s:s + 128], t2[:, s:s + 128], idt[:]
                            )
                        nc.scalar.activation(t2T[:, :, b, :], psT[:], COPY)
                    for wc in range(4):
                        o_ = offs[(c, r, wc)] * 128
                        nc.tensor.matmul(
                            ps2[:],
                            vbt[:, o_:o_ + 128],
                            t2T[:, wc, :, :],
                            start=(k2 == 0),
                            stop=(k2 == n_acc - 1),
                        )
                        k2 += 1
                oT = opool.tile([128, NB * OH], f32, tag="oT")
                nc.scalar.activation(oT[:], ps2[:], COPY)
                nc.scalar.dma_start(
                    out[c], oT[:].rearrange("j (b i) -> j b i", b=NB)
                )

    nc.compile()
    return nc


def kernel(x, weight):
    global LAST
    x = np.ascontiguousarray(np.asarray(x, dtype=np.float32))
    weight = np.asarray(weight, dtype=np.float32)
    assert x.shape == (NB * N_CORES, C, H, W), x.shape

    terms = _filters(weight)
    ranks = tuple(len(t) for t in terms)
    ub_h, vb_h, id_h, offs = _host_tensors(terms)
    nm = ub_h.shape[1] // 128

    import ml_dtypes

    general = sum(ranks) > 6
    vb_h = vb_h.astype(ml_dtypes.bfloat16)
    id_h = id_h.astype(ml_dtypes.bfloat16)
    ub_h = ub_h.astype(ml_dtypes.bfloat16)
    if not general:
        # fast path ships the input in bf16 (DMA-bound kernel: halves HBM)
        x = x.astype(ml_dtypes.bfloat16)

    key = (ranks, general, tuple(sorted(offs.items())))
    if key not in _graph_cache:
        build = _build_graph_general if general else _build_graph
        _graph_cache[key] = build(list(ranks), nm, offs)
    nc = _graph_cache[key]

    in_maps = [
        {"x": x[i * NB:(i + 1) * NB], "ub": ub_h, "vb": vb_h, "ident": id_h}
        for i in range(N_CORES)
    ]

    from concourse.bass_utils import run_bass_kernel_spmd

    trace = bool(int(os.environ.get("BASS_KERNEL_TRACE", "0")))
    LAST = run_bass_kernel_spmd(nc, in_maps, core_ids=list(range(N_CORES)),
                                trace=trace)
    # device emits [C, OH(j), NB(b), OW(i)] per core (bf16 on the fast
    # path) -> upconvert and permute to (b, c, i, j)
    out_t = np.stack([np.asarray(LAST.results[i]["out"], dtype=np.float32)
                      for i in range(N_CORES)], axis=0)
    out_t = out_t.transpose(0, 3, 1, 4, 2).reshape(N_CORES * NB, C, OH, OW)
    return np.ascontiguousarray(out_t)



# revision 12
# speedup vs baseline: 1.1534x; 1.1534x over previous
"""AntiAliasInterpolation2d (depthwise 13x13 gaussian blur + 0.25x bilinear
downsample) as a single stride-4 14x14 depthwise conv, run SPMD on 8 TRN2
NeuronCores (batch-parallel: 4 images per core).

Math: with SCALE=0.25 the bilinear resample picks rows/cols {4k+1, 4k+2} with
weight 0.5 each, so
    out[i,j] = sum_{m,n=0..13} W2[m,n] * x[4i+m-5, 4j+n-5]     (OOB -> 0)
where W2 = 0.25 * (W + W>>row + W>>col + W>>rowcol) is the 13x13 kernel
convolved with a 2x2 box.  W2 is decomposed per-channel by SVD into rank-1
terms (the gaussian is exactly rank 1), each term a 14-tap row filter `a` and
col filter `b`.

Per core, per channel c (4 images b=0..3):
  pass1 (TensorE, fp32r): tmp2[i,w] = sum_r Ut[r,i] * x[r,w]   (banded Ut)
  transpose tmp2 chunks on TensorE (bf16) -> t2T[w,(b,i)]
  pass2 (TensorE, bf16):  psum[j,(b,i)] = sum_w V[w,j] * t2T[w,(b,i)]
The device emits out_t[c,j,b,i]; the host permutes back to [b,c,i,j].
"""

import os

import numpy as np

os.environ.setdefault("MYCRO_LOCAL_CACHE", "1")

N_CORES = 8
NB = 4  # batch shard per core (32 / 8)
C = 3
H = W = 512
OH = OW = 128
KS = 14  # combined stride-4 kernel size
PAD = 5  # x row index = 4*i + m - PAD

LAST = None  # BassKernelResults of the most recent run (for test harness)

_graph_cache = {}


def _filters(weight):
    """Per-channel rank-1 decomposition of the combined 14x14 kernel."""
    Wk = np.asarray(weight, dtype=np.float64).reshape(C, 13, 13)
    W2 = np.zeros((C, KS, KS))
    W2[:, :13, :13] += Wk
    W2[:, 1:, :13] += Wk
    W2[:, :13, 1:] += Wk
    W2[:, 1:, 1:] += Wk
    W2 *= 0.25
    terms = []
    for c in range(C):
        U, S, Vt = np.linalg.svd(W2[c])
        keep = [r for r in range(KS) if S[r] > max(S[0] * 1e-7, 0.0)]
        terms.append([(U[:, r] * S[r], Vt[r].copy()) for r in keep])
    return terms


def _banded(vec):
    """[512, 128] banded matrix M[xr, i] = vec[xr - 4i + PAD] (in-band only)."""
    M = np.zeros((H, OH), np.float32)
    for i in range(OH):
        base = 4 * i - PAD
        for m in range(KS):
            xr = base + m
            if 0 <= xr < H:
                M[xr, i] = vec[m]
    return M


def _host_tensors(terms):
    """Build the flat banded-matrix stacks shipped to the device.

    ub chunk j is Ut[j::4] (row-residue chunking): the x tile stores rows
    4p..4p+3 on partition p (one contiguous 8KB DMA run per partition), so
    K-chunk j of the contraction is x rows {4p+j} paired with Ut[4p+j, i].
    vb stays block-chunked (it contracts over transposed 128-col blocks).
    """
    ub_chunks, vb_chunks = [], []
    index = {}   # (c, r, ch) -> chunk slot (deduped across identical filters)
    seen = {}    # filter fingerprint -> first slot of its 4 chunks
    for c in range(C):
        for r, (a_vec, b_vec) in enumerate(terms[c]):
            fp = (a_vec.tobytes(), b_vec.tobytes())
            if fp in seen:
                base = seen[fp]
                for ch in range(4):
                    index[(c, r, ch)] = base + ch
                continue
            base = len(ub_chunks)
            seen[fp] = base
            Ua = _banded(a_vec)
            Vb = _banded(b_vec)
            for ch in range(4):
                ub_chunks.append(Ua[ch::4])
                vb_chunks.append(Vb[ch * 128:(ch + 1) * 128])
                index[(c, r, ch)] = base + ch
    ub = np.stack(ub_chunks, axis=1).reshape(128, -1).copy()
    vb = np.stack(vb_chunks, axis=1).reshape(128, -1).copy()
    ident = np.eye(128, dtype=np.float32)
    return ub, vb, ident, index


def _build_graph(ranks, nm, offs):
    """Build the per-core Bass graph. `ranks[c]` = #rank-1 terms for channel c;
    `offs` maps (c, r, chunk) -> deduped chunk slot in ub/vb."""
    import concourse.mybir as mybir
    from concourse import bacc, tile

    f32 = mybir.dt.float32
    COPY = mybir.ActivationFunctionType.Copy

    st_dt = mybir.dt.bfloat16
    bf16 = mybir.dt.bfloat16

    nc = bacc.Bacc()
    # x ships as bf16: halves HBM traffic (the kernel is DMA-bound) and
    # runs pass-1 fully in bf16 (FWL weight loads)
    x = nc.declare_dram_parameter("x", [NB, C, H, W], bf16, isOutput=False)
    ub = nc.declare_dram_parameter("ub", [128, nm * 128], bf16, isOutput=False)
    vb = nc.declare_dram_parameter("vb", [128, nm * 128], st_dt, isOutput=False)
    ident = nc.declare_dram_parameter("ident", [128, 128], st_dt, isOutput=False)
    out = nc.declare_dram_parameter("out", [C, OH, NB, OW], st_dt, isOutput=True)

    max_r = max(ranks)

    with tile.TileContext(nc) as tc:
        with (
            tc.tile_pool(name="const", bufs=1) as constp,
            tc.tile_pool(name="xg", bufs=12) as xpool,
            tc.tile_pool(name="t2", bufs=3) as t2pool,
            tc.tile_pool(name="t2T", bufs=2) as tTpool,
            tc.tile_pool(name="oT", bufs=2) as opool,
            tc.tile_pool(name="ps1", bufs=3, space="PSUM") as ps1pool,
            tc.tile_pool(name="psT", bufs=3, space="PSUM") as psTpool,
            tc.tile_pool(name="ps2", bufs=2, space="PSUM") as ps2pool,
        ):
            # consts split across the two HWDGE rings (sync + scalar) so
            # both start streaming immediately
            ubt = constp.tile([128, nm * 128], bf16)
            nc.sync.dma_start(ubt[:], ub[:])
            vbt = constp.tile([128, nm * 128], st_dt)
            nc.scalar.dma_start(vbt[:], vb[:])
            idt = constp.tile([128, 128], st_dt)
            nc.scalar.dma_start(idt[:], ident[:])

            # plane list: (c, b, r) in processing order, grouped by channel
            planes = [(c, b, r) for c in range(C)
                      for b in range(NB) for r in range(ranks[c])]
            n_pl = len(planes)

            # stage state carried across the software pipeline
            xg = {}     # (c, b) -> x tile
            t2s = {}    # idx -> t2 tile
            psTs = {}   # idx -> psT tile
            t2Ts = {}   # c -> t2T group tile
            ps1s = {}   # idx -> ps1 tile

            def s0_dma(idx):  # input DMA (sync ring only: keeping one ring
                # streaming preserves per-DMA latency; splitting rings was
                # measured to starve the pipeline head and re-throttle HAM)
                c, b, r = planes[idx]
                if (c, b) in xg:
                    return
                xt = xpool.tile([128, 4, W], bf16, tag="xg")
                nc.sync.dma_start(
                    xt[:], x[b, c].rearrange("(p k) w -> p k w", k=4)
                )
                xg[(c, b)] = xt

            def s1_mm(idx):  # pass-1 matmuls (PE)
                c, b, r = planes[idx]
                ps1 = ps1pool.tile([128, W], f32, tag="ps1")
                for kc in range(4):
                    o_ = offs[(c, r, kc)] * 128
                    nc.tensor.matmul(
                        ps1[:],
                        ubt[:, o_:o_ + 128],
                        xg[(c, b)][:, kc, :],
                        start=(kc == 0),
                        stop=(kc == 3),
                    )
                ps1s[idx] = ps1

            def s2_copy(idx):  # PSUM -> SBUF bf16 cast (DVE, otherwise idle)
                t2 = t2pool.tile([128, W], st_dt, tag="t2")
                nc.vector.tensor_copy(t2[:], ps1s.pop(idx)[:])
                t2s[idx] = t2

            def s3_transpose(idx):  # PE transposes
                psT = psTpool.tile([128, W], st_dt, tag="psT")
                t2 = t2s.pop(idx)
                for wc in range(4):
                    s = wc * 128
                    nc.tensor.transpose(psT[:, s:s + 128], t2[:, s:s + 128], idt[:])
                psTs[idx] = psT

            def s4_copy(idx):  # PSUM -> grouped SBUF (ACT)
                c, b, r = planes[idx]
                if c not in t2Ts:
                    t2Ts[c] = tTpool.tile([128, max_r, 4, NB, OH], st_dt, tag="t2T", name=f"t2T_{c}")
                nc.scalar.activation(t2Ts[c][:, r, :, b, :], psTs.pop(idx)[:], COPY)

            def s5_chunk(c, b0, nb, ring=None):
                # pass-2 for images [b0, b0+nb) of channel c
                rc_n = ranks[c]
                bs = slice(b0, b0 + nb)
                ps2 = ps2pool.tile([128, nb * OH], f32, tag="ps2", name=f"ps2_{c}_{b0}")
                n_acc = rc_n * 4
                k2 = 0
                for r in range(rc_n):
                    for wc in range(4):
                        o_ = offs[(c, r, wc)] * 128
                        nc.tensor.matmul(
                            ps2[:],
                            vbt[:, o_:o_ + 128],
                            t2Ts[c][:, r, wc, bs, :],
                            start=(k2 == 0),
                            stop=(k2 == n_acc - 1),
                        )
                        k2 += 1
                if b0 + nb == NB:
                    t2Ts.pop(c)
                oT = opool.tile([128, nb * OH], st_dt, tag="oT", name=f"oT_{c}_{b0}")
                nc.scalar.activation(oT[:], ps2[:], COPY)
                (ring or nc.scalar).dma_start(
                    out[c][:, bs, :], oT[:].rearrange("j (b i) -> j b i", b=nb)
                )

            # warm the PE HAM clock-gate with throwaway matmuls while the
            # input stream fills.  The weights come from a memset tile, not
            # a DMA'd const, so the warmup starts at sequencer boot instead
            # of waiting ~3.5us for the first DMA completion.
            warm = constp.tile([128, 512], bf16)
            nc.gpsimd.memset(warm[:], 0.5)
            wps = ps1pool.tile([128, 512], f32, tag="ps1", name="wps")
            for _ in range(14):
                nc.tensor.matmul(wps[:], warm[:, 0:128], warm[:],
                                 start=True, stop=True)

            # prefetch all input DMAs up front (12 bufs cover all planes)
            done_dma = set()
            for idx in range(n_pl):
                c, b, r = planes[idx]
                if (c, b) not in done_dma:
                    s0_dma(idx)
                    done_dma.add((c, b))

            # software pipeline: keep the PE stream dense
            LAG = 1
            # plane idx -> (c, b0, nb, ring) pass-2 chunk ready at this plane
            half_last = {}
            seen = {}
            for idx, (c, b, r) in enumerate(planes):
                seen.setdefault(c, []).append(idx)
            last_c = max(seen)
            for c, idxs in seen.items():
                rc_n = ranks[c]
                half_last[idxs[2 * rc_n - 1]] = [(c, 0, 2, None)]
                if c == last_c:
                    # final channel: split the tail chunk so the last
                    # arriving image has the shortest chain to HBM, and
                    # send the final output on the (by then idle) sync ring
                    half_last[idxs[3 * rc_n - 1]] = [(c, 2, 1, None)]
                    half_last[idxs[4 * rc_n - 1]] = [(c, 3, 1, nc.sync)]
                else:
                    half_last[idxs[4 * rc_n - 1]] = [(c, 2, 2, None)]
            for idx in range(n_pl + LAG + 1):
                if idx < n_pl:
                    s1_mm(idx)
                if 0 <= idx - LAG < n_pl:
                    j = idx - LAG
                    s2_copy(j)
                    s3_transpose(j)
                    s4_copy(j)
                    for chunk in half_last.get(j, ()):
                        s5_chunk(*chunk)

    nc.compile()
    return nc


def _build_graph_general(ranks, nm, offs):
    """Fallback for high-rank (non-separable) kernels: bf16 end-to-end,
    rank-major loop, small buffers. Correctness over speed — the shipped
    Gaussian weight is rank-1 per channel and never takes this path."""
    import concourse.mybir as mybir
    from concourse import bacc, tile

    f32 = mybir.dt.float32
    bf16 = mybir.dt.bfloat16
    COPY = mybir.ActivationFunctionType.Copy

    nc = bacc.Bacc()
    x = nc.declare_dram_parameter("x", [NB, C, H, W], f32, isOutput=False)
    ub = nc.declare_dram_parameter("ub", [128, nm * 128], bf16, isOutput=False)
    vb = nc.declare_dram_parameter("vb", [128, nm * 128], bf16, isOutput=False)
    ident = nc.declare_dram_parameter("ident", [128, 128], bf16, isOutput=False)
    out = nc.declare_dram_parameter("out", [C, OH, NB, OW], f32, isOutput=True)

    with tile.TileContext(nc) as tc:
        with (
            tc.tile_pool(name="const", bufs=1) as constp,
            tc.tile_pool(name="xg", bufs=6) as xpool,
            tc.tile_pool(name="xb", bufs=6) as xbpool,
            tc.tile_pool(name="t2", bufs=3) as t2pool,
            tc.tile_pool(name="t2T", bufs=3) as tTpool,
            tc.tile_pool(name="oT", bufs=2) as opool,
            tc.tile_pool(name="ps1", bufs=2, space="PSUM") as ps1pool,
            tc.tile_pool(name="psT", bufs=2, space="PSUM") as psTpool,
            tc.tile_pool(name="ps2", bufs=2, space="PSUM") as ps2pool,
        ):
            ubt = constp.tile([128, nm * 128], bf16)
            nc.sync.dma_start(ubt[:], ub[:])
            vbt = constp.tile([128, nm * 128], bf16)
            nc.sync.dma_start(vbt[:], vb[:])
            idt = constp.tile([128, 128], bf16)
            nc.sync.dma_start(idt[:], ident[:])

            for c in range(C):
                rc_n = ranks[c]
                xb = []
                for b in range(NB):
                    xt = xpool.tile([128, 4, W], f32, tag="xg")
                    nc.sync.dma_start(
                        xt[:], x[b, c].rearrange("(p k) w -> p k w", k=4)
                    )
                    xc = xbpool.tile([128, 4, W], bf16, tag="xb")
                    nc.vector.tensor_copy(xc[:], xt[:])
                    xb.append(xc)

                ps2 = ps2pool.tile([128, NB * OH], f32, tag="ps2")
                n_acc = rc_n * 4
                k2 = 0
                for r in range(rc_n):
                    t2T = tTpool.tile([128, 4, NB, OH], bf16, tag="t2T",
                                      name=f"t2Tg_{c}_{r}")
                    for b in range(NB):
                        ps1 = ps1pool.tile([128, W], f32, tag="ps1")
                        for kc in range(4):
                            o_ = offs[(c, r, kc)] * 128
                            nc.tensor.matmul(
                                ps1[:],
                                ubt[:, o_:o_ + 128],
                                xb[b][:, kc, :],
                                start=(kc == 0),
                                stop=(kc == 3),
                            )
                        t2 = t2pool.tile([128, W], bf16, tag="t2")
                        nc.scalar.activation(t2[:], ps1[:], COPY)
                        psT = psTpool.tile([128, W], bf16, tag="psT")
                        for wc in range(4):
                            s = wc * 128
                            nc.tensor.transpose(
                                psT[:, s:s + 128], t2[:, s:s + 128], idt[:]
                            )
                        nc.scalar.activation(t2T[:, :, b, :], psT[:], COPY)
                    for wc in range(4):
                        o_ = offs[(c, r, wc)] * 128
                        nc.tensor.matmul(
                            ps2[:],
                            vbt[:, o_:o_ + 128],
                            t2T[:, wc, :, :],
                            start=(k2 == 0),
                            stop=(k2 == n_acc - 1),
                        )
                        k2 += 1
                oT = opool.tile([128, NB * OH], f32, tag="oT")
                nc.scalar.activation(oT[:], ps2[:], COPY)
                nc.scalar.dma_start(
                    out[c], oT[:].rearrange("j (b i) -> j b i", b=NB)
                )

    nc.compile()
    return nc


def kernel(x, weight):
    global LAST
    x = np.ascontiguousarray(np.asarray(x, dtype=np.float32))
    weight = np.asarray(weight, dtype=np.float32)
    assert x.shape == (NB * N_CORES, C, H, W), x.shape

    terms = _filters(weight)
    ranks = tuple(len(t) for t in terms)
    ub_h, vb_h, id_h, offs = _host_tensors(terms)
    nm = ub_h.shape[1] // 128

    import ml_dtypes

    general = sum(ranks) > 6
    vb_h = vb_h.astype(ml_dtypes.bfloat16)
    id_h = id_h.astype(ml_dtypes.bfloat16)
    ub_h = ub_h.astype(ml_dtypes.bfloat16)
    if not general:
        # fast path ships the input in bf16 (DMA-bound kernel: halves HBM)
        x = x.astype(ml_dtypes.bfloat16)

    key = (ranks, general, tuple(sorted(offs.items())))
    if key not in _graph_cache:
        build = _build_graph_general if general else _build_graph
        _graph_cache[key] = build(list(ranks), nm, offs)
    nc = _graph_cache[key]

    in_maps = [
        {"x": x[i * NB:(i + 1) * NB], "ub": ub_h, "vb": vb_h, "ident": id_h}
        for i in range(N_CORES)
    ]

    from concourse.bass_utils import run_bass_kernel_spmd

    trace = bool(int(os.environ.get("BASS_KERNEL_TRACE", "0")))
    LAST = run_bass_kernel_spmd(nc, in_maps, core_ids=list(range(N_CORES)),
                                trace=trace)
    # device emits [C, OH(j), NB(b), OW(i)] per core (bf16 on the fast
    # path) -> upconvert and permute to (b, c, i, j)
    out_t = np.stack([np.asarray(LAST.results[i]["out"], dtype=np.float32)
                      for i in range(N_CORES)], axis=0)
    out_t = out_t.transpose(0, 3, 1, 4, 2).reshape(N_CORES * NB, C, OH, OW)
    return np.ascontiguousarray(out_t)

